# revision 2
# baseline (speedup 1.0000x reference)
"""Trainium2 Bass kernel for nn_ETypePromptModel: logits = einsum('bpd,cpd->bc').

Equivalent to X @ W.T with X=[B, L*D]=[16384, 256], W=[C, L*D]=[4096, 256].
Data-parallel over B across 8 NeuronCores; label2embed replicated.

Per-core plan (B_LOC=2048):
  - PE-transpose W (64 [128,128] tiles) and X (32 tiles) into K-major SBUF
    layout (fp32 has no DMA-transpose path).
  - 256 float32r matmuls ([128k x 128b] stationary, [128k x 512c] moving),
    K=256 accumulated over 2 PSUM passes.
  - PSUM -> SBUF copy on ACT/DVE, 16 x 2MB HWDGE DMA writes of the
    [2048, 4096] fp32 output slice.
"""

import sys

import numpy as np

sys.path.insert(0, "/opt/trn_rl_repo")

B, C, L, D = 16384, 4096, 2, 128
N_CORES = 8
B_LOC = B // N_CORES  # 2048
P = 128
N_TILE = 512  # moving free dim per matmul
M_TILES = B_LOC // P  # 16
N_TILES = C // N_TILE  # 8
C_TILES = C // P  # 32
N_GROUP = 4  # PSUM banks used concurrently by matmul accumulation

_CACHE = {}
PROFILE = False
TRACE_ALL_CORES = False
LAST_RESULT = None


def _build():
    import concourse.mybir as mybir
    import concourse.tile as tile
    from concourse import bacc
    from concourse.masks import make_identity

    f32 = mybir.dt.float32
    f32r = mybir.dt.float32r

    nc = bacc.Bacc(
        "TRN2",
        target_bir_lowering=False,
        debug=False,
        enable_asserts=False,
        num_devices=N_CORES,
    )

    x_dram = nc.dram_tensor("batchs", [B_LOC, L, D], f32, kind="ExternalInput").ap()
    w_dram = nc.dram_tensor("label2embed", [C, L, D], f32, kind="ExternalInput").ap()
    out_dram = nc.dram_tensor("out", [B_LOC, C], f32, kind="ExternalOutput").ap()

    with tile.TileContext(nc) as tc:
        with (
            tc.tile_pool(name="const", bufs=1) as const_pool,
            tc.tile_pool(name="wt", bufs=1) as wt_pool,
            tc.tile_pool(name="stage", bufs=4) as stage_pool,
            tc.tile_pool(name="xt", bufs=3) as xt_pool,
            tc.tile_pool(name="osb", bufs=3) as out_pool,
            tc.tile_pool(name="pst", bufs=2, space="PSUM") as psum_t,
            tc.tile_pool(name="psm", bufs=4, space="PSUM") as psum_mm,
        ):
            ident = const_pool.tile([P, P], f32, name="ident")
            make_identity(nc, ident)

            # W.T in SBUF: wt[d, p, c] = W[c, p, d]  (32 KB/partition)
            wt = wt_pool.tile([P, L, C], f32r, name="wt")
            for ct in range(C_TILES):
                w_nat = stage_pool.tile([P, L, D], f32, tag="w_nat", name="w_nat")
                nc.sync.dma_start(w_nat, w_dram[ct * P : (ct + 1) * P])
                for p in range(L):
                    ps = psum_t.tile([P, P], f32, tag="tps", name="tps_w")
                    nc.tensor.transpose(ps, w_nat[:, p, :], ident)
                    nc.any.tensor_copy(out=wt[:, p, ct * P : (ct + 1) * P], in_=ps)

            for mt in range(M_TILES):
                x_nat = stage_pool.tile([P, L, D], f32, tag="x_nat", name="x_nat")
                nc.sync.dma_start(x_nat, x_dram[mt * P : (mt + 1) * P])
                # X.T tile: xt[d, p, b] = X[b, p, d] for this 128-row b block
                xt = xt_pool.tile([P, L, P], f32r, tag="xt", name="xt")
                for p in range(L):
                    ps = psum_t.tile([P, P], f32, tag="tps", name="tps_x")
                    nc.tensor.transpose(ps, x_nat[:, p, :], ident)
                    nc.any.tensor_copy(out=xt[:, p, :], in_=ps)

                out_sb = out_pool.tile([P, C], f32, tag="osb", name="out_sb")
                for ng in range(N_TILES // N_GROUP):
                    pms = [
                        psum_mm.tile([P, N_TILE], f32, tag="pmm", name="pmm")
                        for _ in range(N_GROUP)
                    ]
                    # p outer: stationary operand xt[:, p, :] reused across
                    # the N_GROUP matmuls (amortizes weight load).
                    for p in range(L):
                        for j in range(N_GROUP):
                            nt = ng * N_GROUP + j
                            nc.tensor.matmul(
                                pms[j],
                                xt[:, p, :],
                                wt[:, p, nt * N_TILE : (nt + 1) * N_TILE],
                                start=(p == 0),
                                stop=(p == L - 1),
                            )
                    for j in range(N_GROUP):
                        nt = ng * N_GROUP + j
                        nc.any.tensor_copy(
                            out=out_sb[:, nt * N_TILE : (nt + 1) * N_TILE],
                            in_=pms[j],
                        )
                nc.sync.dma_start(out_dram[mt * P : (mt + 1) * P], out_sb)

    nc.compile()
    return nc


def kernel(batchs, label2embed):
    global LAST_RESULT
    from concourse.bass_utils import run_bass_kernel_spmd

    if "nc" not in _CACHE:
        _CACHE["nc"] = _build()
    nc = _CACHE["nc"]

    batchs = np.ascontiguousarray(batchs, dtype=np.float32)
    label2embed = np.ascontiguousarray(label2embed, dtype=np.float32)
    assert batchs.shape == (B, L, D) and label2embed.shape == (C, L, D)

    in_maps = [
        {
            "batchs": batchs[c * B_LOC : (c + 1) * B_LOC],
            "label2embed": label2embed,
        }
        for c in range(N_CORES)
    ]
    res = run_bass_kernel_spmd(
        nc,
        in_maps,
        core_ids=list(range(N_CORES)),
        trace=PROFILE,
        trace_cores=list(range(N_CORES)) if (PROFILE and TRACE_ALL_CORES) else None,
    )
    LAST_RESULT = res
    return np.concatenate([r["out"] for r in res.results], axis=0)


# revision 5
# speedup vs baseline: 1.2424x; 1.2424x over previous
"""Trainium2 Bass kernel for nn_ETypePromptModel: logits = einsum('bpd,cpd->bc').

Equivalent to X @ W.T with X=[B, L*D]=[16384, 256], W=[C, L*D]=[4096, 256].
Data-parallel over B across 8 NeuronCores; label2embed replicated.

Per-core plan (B_LOC=2048):
  - Load X (1 DMA, 2 MB) and W (4 DMAs, 4 MB) up front at line rate.
  - PE-transpose X (32 tiles) and W (64 tiles) into K-major float32r SBUF
    layout; 4 transposes batched per PSUM bank -> one [128,512] copy each.
  - 256 float32r matmuls ([128k x 128b] stationary, [128k x 512c] moving),
    K=256 accumulated over 2 PSUM passes, 6 PSUM banks in rotation.
  - PSUM -> SBUF copies alternate Vector/Scalar engines; 16 x 2MB HWDGE
    DMA writes of the [2048, 4096] fp32 output slice.
"""

import sys

import numpy as np

sys.path.insert(0, "/opt/trn_rl_repo")

B, C, L, D = 16384, 4096, 2, 128
N_CORES = 8
B_LOC = B // N_CORES  # 2048
P = 128
N_TILE = 512  # moving free dim per matmul
M_TILES = B_LOC // P  # 16
N_TILES = C // N_TILE  # 8
C_TILES = C // P  # 32
N_GROUP = 4  # matmul accumulation group width (PSUM banks)
W_CHUNKS = 4  # input-load chunks for W
C_HALF = C // 2

_CACHE = {}
PROFILE = False
TRACE_ALL_CORES = False
LAST_RESULT = None


def _build():
    import concourse.mybir as mybir
    import concourse.tile as tile
    from concourse import bacc
    from concourse.masks import make_identity

    f32 = mybir.dt.float32
    f32r = mybir.dt.float32r

    nc = bacc.Bacc(
        "TRN2",
        target_bir_lowering=False,
        debug=False,
        enable_asserts=False,
        num_devices=N_CORES,
    )

    x_dram = nc.dram_tensor("batchs", [B_LOC, L, D], f32, kind="ExternalInput").ap()
    w_dram = nc.dram_tensor("label2embed", [C, L, D], f32, kind="ExternalInput").ap()
    out_dram = nc.dram_tensor("out", [B_LOC, C], f32, kind="ExternalOutput").ap()

    with tile.TileContext(nc) as tc:
        with (
            tc.tile_pool(name="const", bufs=1) as const_pool,
            tc.tile_pool(name="big", bufs=1) as big_pool,
            tc.tile_pool(name="osb", bufs=4) as out_pool,
            tc.tile_pool(name="pst", bufs=2, space="PSUM") as psum_t,
            tc.tile_pool(name="psm", bufs=6, space="PSUM") as psum_mm,
        ):
            ident = const_pool.tile([P, P], f32, name="ident")
            make_identity(nc, ident)

            _cp = [0]

            def copy(out_ap, in_ap):
                if _cp[0] % 2 == 0:
                    nc.vector.tensor_copy(out=out_ap, in_=in_ap)
                else:
                    nc.scalar.copy(out_ap, in_ap)
                _cp[0] += 1

            # ---- bulk input loads (line-rate, few big DMAs) ----
            x_stage = big_pool.tile([P, M_TILES, L, D], f32, name="x_stage")
            nc.sync.dma_start(
                x_stage, x_dram.rearrange("(mo bi) p d -> bi mo p d", bi=P)
            )
            CO = C_TILES // W_CHUNKS  # 8 c-tiles per chunk
            w_stages = []
            for ci in range(W_CHUNKS):
                w_st = big_pool.tile([P, CO, L, D], f32, name=f"w_stage{ci}")
                nc.sync.dma_start(
                    w_st,
                    w_dram[ci * CO * P : (ci + 1) * CO * P].rearrange(
                        "(co bi) p d -> bi co p d", bi=P
                    ),
                )
                w_stages.append(w_st)

            # ---- transpose prologue ----
            # X.T: xt[d, p, b] = X[b, p, d]; 4 transposes per PSUM bank,
            # then one [128, 512] strided copy out.
            xt = big_pool.tile([P, L, B_LOC], f32r, name="xt")
            for mo2 in range(M_TILES // 2):
                ps = psum_t.tile([P, 2, L, P], f32, tag="tps", name="tps_x")
                for m1 in range(2):
                    for p in range(L):
                        nc.tensor.transpose(
                            ps[:, m1, p, :], x_stage[:, mo2 * 2 + m1, p, :], ident
                        )
                copy(
                    xt[:, :, mo2 * 2 * P : (mo2 * 2 + 2) * P].rearrange(
                        "d p (m b) -> d p m b", m=2
                    ),
                    ps.rearrange("d m p b -> d p m b"),
                )

            # W.T in two halves so early matmuls start before all of W lands:
            # wt_halves[h][d, p, c'] = W[h*2048 + c', p, d]
            wt_halves = [
                big_pool.tile([P, L, C_HALF], f32r, name=f"wt{h}") for h in range(2)
            ]
            for ci in range(W_CHUNKS):
                w_st = w_stages[ci]
                wt = wt_halves[ci // 2]
                base = (ci % 2) * CO * P  # c offset within the half
                for co2 in range(CO // 2):
                    ps = psum_t.tile([P, 2, L, P], f32, tag="tps", name="tps_w")
                    for m1 in range(2):
                        for p in range(L):
                            nc.tensor.transpose(
                                ps[:, m1, p, :], w_st[:, co2 * 2 + m1, p, :], ident
                            )
                    copy(
                        wt[
                            :, :, base + co2 * 2 * P : base + (co2 * 2 + 2) * P
                        ].rearrange("d p (m b) -> d p m b", m=2),
                        ps.rearrange("d m p b -> d p m b"),
                    )

            # ---- main matmul stream ----
            for mt in range(M_TILES):
                out_sb = out_pool.tile([P, C], f32, tag="osb", name="out_sb")
                for ng in range(N_TILES // N_GROUP):
                    pms = [
                        psum_mm.tile([P, N_TILE], f32, tag="pmm", name="pmm")
                        for _ in range(N_GROUP)
                    ]
                    # p outer: stationary operand xt slice reused across the
                    # N_GROUP matmuls (amortizes weight load).
                    for p in range(L):
                        for j in range(N_GROUP):
                            nt = ng * N_GROUP + j
                            wt = wt_halves[nt * N_TILE // C_HALF]
                            noff = nt * N_TILE % C_HALF
                            nc.tensor.matmul(
                                pms[j],
                                xt[:, p, mt * P : (mt + 1) * P],
                                wt[:, p, noff : noff + N_TILE],
                                start=(p == 0),
                                stop=(p == L - 1),
                            )
                    for j in range(N_GROUP):
                        nt = ng * N_GROUP + j
                        copy(out_sb[:, nt * N_TILE : (nt + 1) * N_TILE], pms[j])
                nc.sync.dma_start(out_dram[mt * P : (mt + 1) * P], out_sb)

    nc.compile()
    return nc


def kernel(batchs, label2embed):
    global LAST_RESULT
    from concourse.bass_utils import run_bass_kernel_spmd

    if "nc" not in _CACHE:
        _CACHE["nc"] = _build()
    nc = _CACHE["nc"]

    batchs = np.ascontiguousarray(batchs, dtype=np.float32)
    label2embed = np.ascontiguousarray(label2embed, dtype=np.float32)
    assert batchs.shape == (B, L, D) and label2embed.shape == (C, L, D)

    in_maps = [
        {
            "batchs": batchs[c * B_LOC : (c + 1) * B_LOC],
            "label2embed": label2embed,
        }
        for c in range(N_CORES)
    ]
    res = run_bass_kernel_spmd(
        nc,
        in_maps,
        core_ids=list(range(N_CORES)),
        trace=PROFILE,
        trace_cores=list(range(N_CORES)) if (PROFILE and TRACE_ALL_CORES) else None,
    )
    LAST_RESULT = res
    return np.concatenate([r["out"] for r in res.results], axis=0)
